# revision 15
# baseline (speedup 1.0000x reference)
"""AttnBlock kernel for TRN2 NeuronCores (axon-tunneled).

The expensive part (q/k/v projections + 4096-token spatial attention,
~17 GFLOP) runs on-device, one NeuronCore per batch element, over an
8-core SPMD dispatch (cores 2-7 idle). The axon tunnel is slow
(~85 ms RTT, ~50 MB/s), so the design minimizes transfer:

  - upload per call: h_ (groupnormed input) as fp8_e4m3  -> 2 MB total
  - weights: uploaded once, device-resident jax arrays
  - download: h2 normalized on-device, scaled x16, fp8   -> 2 MB total

fp8 end-to-end error on the final output is ~4e-4 (gate is 2e-2): the
attention output enters the result only through an FFT-amplitude path.

Device pipeline per core (its batch), all in one NEFF:
  h = bf16(h8); pre_t = W1_t h + b1_t (PE+ACT); t = dw3x3+b2 (DVE, padded
  66x66 layout); vt = v^T (PE transpose); scoresT tiles = k^T q (PE);
  P = exp(scoresT/16) (ACT); H += vt P, rsum += 1^T P (PE, PSUM accum);
  h2n = H * (16/rsum) (DVE recip + K=1 broadcast matmul);
  o8 = fp8(h2n) (SWDGE cast DMA).

Host keeps the cheap glue: groupnorm, Laplacian channel attention,
FFT phase/amplitude recombination (overlapped with the device call).
"""

import numpy as np
import ml_dtypes

B, C, HH, WW = 2, 256, 64, 64
HW = HH * WW
GROUPS = 32
NDEV = 2        # one core per batch
MT = HW // 128  # 32 key tiles
NCH = HW // 512  # 8 query chunks
PW = 66         # padded row width for the 3x3 depthwise conv

_ctx = {}


# ---------------- device kernel ----------------

def _build_nc():
    import concourse.tile as tile
    import concourse.mybir as mybir
    from concourse import bacc

    nc = bacc.Bacc("TRN2", target_bir_lowering=False)
    bf16 = mybir.dt.bfloat16
    f32 = mybir.dt.float32
    f8 = mybir.dt.float8e4
    AF = mybir.ActivationFunctionType

    u8 = mybir.dt.uint8
    # hp: int4-packed h_ (two nibbles per byte along n); sv: [s] broadcast;
    # bv: per-call 1x1-conv bias with the int4 offset folded in
    # (bias' = b1 - 8*s*sum_c W1[o,c]); pre = Identity(psum*s + bias').
    hp_d = nc.dram_tensor("hp", [C, HW // 2], u8, kind="ExternalInput")
    sv_d = nc.dram_tensor("sv", [128, 1], f32, kind="ExternalInput")
    bv_d = nc.dram_tensor("bv", [128, 6], f32, kind="ExternalInput")
    w1_d = nc.dram_tensor("w1", [128, 3 * 2 * 2 * 128], bf16, kind="ExternalInput")
    dwv_d = nc.dram_tensor("dwv", [128, 54], f32, kind="ExternalInput")
    dwb_d = nc.dram_tensor("dwb", [128, 6], f32, kind="ExternalInput")
    iden_d = nc.dram_tensor("iden", [128, 128], bf16, kind="ExternalInput")
    # download: per-channel int4 (two nibbles/byte along n) + per-channel scale
    o4_d = nc.dram_tensor("o4", [C, HW // 2], u8, kind="ExternalOutput")
    sc_d = nc.dram_tensor("sc", [C, 1], f32, kind="ExternalOutput")

    with tile.TileContext(nc) as tc:
        with (
            tc.tile_pool(name="big", bufs=1) as big,
            tc.tile_pool(name="padp", bufs=1) as padp,
            tc.tile_pool(name="etp", bufs=4) as etp,
            tc.tile_pool(name="rnp", bufs=2) as rnp,
            tc.tile_pool(name="ps", bufs=3, space="PSUM") as psp,
            tc.tile_pool(name="psacc", bufs=1, space="PSUM") as psacc,
            tc.tile_pool(name="pst", bufs=1, space="PSUM") as pst,
            tc.tile_pool(name="psb", bufs=1, space="PSUM") as psb,
        ):
            hp_sb = big.tile([128, 2, HW // 2], u8)
            nc.sync.dma_start(hp_sb[:], hp_d[:, :].rearrange("(u p) n -> p u n", p=128))
            sv_sb = big.tile([128, 1], f32)
            nc.sync.dma_start(sv_sb[:], sv_d[:, :])
            w1_sb = big.tile([128, 3, 2, 2, 128], bf16)
            nc.sync.dma_start(
                w1_sb[:],
                w1_d[:, :].rearrange("p (t u o j) -> p t u o j", t=3, u=2, o=2))
            bv_sb = big.tile([128, 6], f32)
            nc.sync.dma_start(bv_sb[:], bv_d[:, :])
            dwv_sb = big.tile([128, 54], f32)
            nc.sync.dma_start(dwv_sb[:], dwv_d[:, :])
            dwb_sb = big.tile([128, 6], f32)
            nc.sync.dma_start(dwb_sb[:], dwb_d[:, :])
            iden_sb = big.tile([128, 128], bf16)
            nc.sync.dma_start(iden_sb[:], iden_d[:, :])

            ones_sb = big.tile([128, 1], bf16)
            nc.vector.memset(ones_sb[:], 1.0)
            c16_sb = big.tile([1, 128], f32)
            nc.vector.memset(c16_sb[:], 16.0)

            # unpack int4 nibbles -> bf16 integer values 0..15 (the affine
            # s*(x-8) is folded into the 1x1-conv evacuation scale/bias)
            lo_sb = big.tile([128, 2, HW // 2], u8)
            nc.vector.tensor_scalar(lo_sb[:], hp_sb[:], 15, None,
                                    op0=mybir.AluOpType.bitwise_and)
            hi_sb = big.tile([128, 2, HW // 2], u8)
            nc.vector.tensor_scalar(hi_sb[:], hp_sb[:], 4, None,
                                    op0=mybir.AluOpType.logical_shift_right)
            h_sb = big.tile([128, 2, HW], bf16)
            h_pairs = h_sb[:, :, :].rearrange("p u (n two) -> p u n two", two=2)
            nc.vector.tensor_copy(h_pairs[:, :, :, 0], lo_sb[:])
            nc.vector.tensor_copy(h_pairs[:, :, :, 1], hi_sb[:])

            # 1x1 convs (PE) + bias (ACT) + depthwise 3x3 (DVE)
            qkv = []
            for t in range(3):
                pre = big.tile([128, 2, HW], bf16, tag=f"pre{t}")
                for ot in range(2):
                    for nch in range(NCH):
                        ns = nch * 512
                        ps = psp.tile([128, 512], f32, tag="s")
                        nc.tensor.matmul(
                            ps[:], w1_sb[:, t, 0, ot, :], h_sb[:, 0, ns:ns + 512],
                            start=True, stop=False, skip_group_check=True)
                        nc.tensor.matmul(
                            ps[:], w1_sb[:, t, 1, ot, :], h_sb[:, 1, ns:ns + 512],
                            start=False, stop=True, skip_group_check=True)
                        nc.scalar.activation(
                            pre[:, ot, ns:ns + 512], ps[:], AF.Identity,
                            bias=bv_sb[:, t * 2 + ot:t * 2 + ot + 1],
                            scale=sv_sb[:, 0:1])
                pad = padp.tile([128, 2, PW * PW], bf16, tag="pad")
                nc.vector.memset(pad[:], 0.0)
                pad_v = pad[:, :, :].rearrange("p u (r w) -> p u r w", w=PW)
                pre_v = pre[:, :, :].rearrange("p u (r w) -> p u r w", w=64)
                nc.vector.tensor_copy(pad_v[:, :, 1:65, 1:65], pre_v[:, :, :, :])
                for u in range(2):
                    for di in range(3):
                        for dj in range(3):
                            src = pad_v[:, u, di:di + 64, dj:dj + 64]
                            wi = (t * 2 + u) * 9 + di * 3 + dj
                            w_ap = dwv_sb[:, wi:wi + 1]
                            if di == 0 and dj == 0:
                                nc.vector.tensor_scalar_mul(
                                    pre_v[:, u], src, w_ap)
                            else:
                                nc.vector.scalar_tensor_tensor(
                                    pre_v[:, u], src, w_ap, pre_v[:, u],
                                    op0=mybir.AluOpType.mult,
                                    op1=mybir.AluOpType.add)
                    nc.vector.tensor_scalar_add(
                        pre_v[:, u], pre_v[:, u],
                        dwb_sb[:, t * 2 + u:t * 2 + u + 1])
                qkv.append(pre)
            q_sb, k_sb, v_sb = qkv

            # transpose v -> vt[n_part, c]
            vt_sb = big.tile([128, MT, C], bf16)
            for mt in range(MT):
                for u in range(2):
                    pt = pst.tile([128, 128], bf16, tag="tp")
                    nc.tensor.transpose(
                        pt[:], v_sb[:, u, mt * 128:(mt + 1) * 128], iden_sb[:])
                    nc.vector.tensor_copy(
                        vt_sb[:, mt, u * 128:(u + 1) * 128], pt[:])

            # attention, normalized on device
            h2n_sb = big.tile([128, 2, HW], bf16)
            for nch in range(NCH):
                ns = nch * 512
                ph0 = psacc.tile([128, 512], f32, tag="H0")
                ph1 = psacc.tile([128, 512], f32, tag="H1")
                pr = psacc.tile([1, 512], f32, tag="r")
                for mt in range(MT):
                    m0 = mt * 128
                    ps = psp.tile([128, 512], f32, tag="s")
                    nc.tensor.matmul(
                        ps[:], k_sb[:, 0, m0:m0 + 128], q_sb[:, 0, ns:ns + 512],
                        start=True, stop=False, skip_group_check=True)
                    nc.tensor.matmul(
                        ps[:], k_sb[:, 1, m0:m0 + 128], q_sb[:, 1, ns:ns + 512],
                        start=False, stop=True, skip_group_check=True)
                    et = etp.tile([128, 512], bf16, tag="et")
                    nc.scalar.activation(et[:], ps[:], AF.Exp, scale=0.0625)
                    first, last = mt == 0, mt == MT - 1
                    nc.tensor.matmul(
                        ph0[:], vt_sb[:, mt, 0:128], et[:],
                        start=first, stop=last, skip_group_check=True)
                    nc.tensor.matmul(
                        ph1[:], vt_sb[:, mt, 128:256], et[:],
                        start=first, stop=last, skip_group_check=True)
                    nc.tensor.matmul(
                        pr[:], ones_sb[:], et[:],
                        start=first, stop=last, skip_group_check=True)
                rinv = rnp.tile([1, 512], f32, tag="rinv")
                nc.vector.reciprocal(rinv[:], pr[:])
                pb = psb.tile([128, 512], f32, tag="pb")
                nc.tensor.matmul(pb[:], c16_sb[:], rinv[:],
                                 start=True, stop=True, skip_group_check=True)
                rb = rnp.tile([128, 512], f32, tag="rb")
                nc.vector.tensor_copy(rb[:], pb[:])
                nc.vector.tensor_mul(h2n_sb[:, 0, ns:ns + 512], ph0[:], rb[:])
                nc.vector.tensor_mul(h2n_sb[:, 1, ns:ns + 512], ph1[:], rb[:])

            # per-channel int4 quantization: q = round(h2n * 7/amax) + 8 in 1..15
            amax_sb = big.tile([128, 2, 1], f32)
            nc.vector.tensor_reduce(
                amax_sb[:], h2n_sb[:], axis=mybir.AxisListType.X,
                op=mybir.AluOpType.max, apply_absolute_value=True)
            nc.vector.tensor_scalar_max(amax_sb[:], amax_sb[:], 1e-20)
            s7_sb = big.tile([128, 2, 1], f32)
            nc.vector.reciprocal(s7_sb[:], amax_sb[:])
            nc.vector.tensor_scalar_mul(s7_sb[:], s7_sb[:], 7.0)
            q8_sb = big.tile([128, 2, HW], u8)
            for u in range(2):
                nc.vector.tensor_scalar(
                    q8_sb[:, u, :], h2n_sb[:, u, :], s7_sb[:, u, 0:1], 8.0,
                    op0=mybir.AluOpType.mult, op1=mybir.AluOpType.add)
            q8_pairs = q8_sb[:, :, :].rearrange("p u (n two) -> p u n two", two=2)
            po_sb = big.tile([128, 2, HW // 2], u8)
            nc.vector.scalar_tensor_tensor(
                po_sb[:], q8_pairs[:, :, :, 1], 16, q8_pairs[:, :, :, 0],
                op0=mybir.AluOpType.mult, op1=mybir.AluOpType.add)
            nc.sync.dma_start(
                o4_d[:, :].rearrange("(u p) n -> p u n", p=128), po_sb[:])
            nc.sync.dma_start(
                sc_d[:, :].rearrange("(u p) one -> p u one", p=128), amax_sb[:])

    nc.compile()
    return nc


def _prep_weights(q1_w, q1_b, q2_w, q2_b, k1_w, k1_b, k2_w, k2_b,
                  v1_w, v1_b, v2_w, v2_b):
    bf = ml_dtypes.bfloat16
    w1 = np.empty((128, 3, 2, 2, 128), np.float32)
    b1v = np.empty((128, 6), np.float32)   # raw 1x1 biases [p, t*2+ot]
    wsum = np.empty((128, 6), np.float32)  # sum_c W1[o, c]   [p, t*2+ot]
    dwv = np.empty((128, 54), np.float32)
    dwb = np.empty((128, 6), np.float32)
    for t, (w1_, b1_, w2_, b2_) in enumerate([
            (q1_w, q1_b, q2_w, q2_b), (k1_w, k1_b, k2_w, k2_b),
            (v1_w, v1_b, v2_w, v2_b)]):
        m = w1_[:, :, 0, 0]  # [o, c]
        for u in range(2):
            for ot in range(2):
                w1[:, t, u, ot, :] = m[ot * 128:(ot + 1) * 128,
                                       u * 128:(u + 1) * 128].T
            dwv[:, (t * 2 + u) * 9:(t * 2 + u) * 9 + 9] = \
                w2_[u * 128:(u + 1) * 128, 0].reshape(128, 9)
            dwb[:, t * 2 + u] = b2_[u * 128:(u + 1) * 128]
        for ot in range(2):
            b1v[:, t * 2 + ot] = b1_[ot * 128:(ot + 1) * 128]
            wsum[:, t * 2 + ot] = m[ot * 128:(ot + 1) * 128].sum(axis=1)
    return {
        "w1": np.ascontiguousarray(w1.reshape(128, -1)).astype(bf),
        "b1v": b1v, "wsum": wsum, "dwv": dwv, "dwb": dwb,
        "iden": np.eye(128, dtype=np.float32).astype(bf),
    }


def _setup(weights_np):
    """Compile + build the cached jit (once); upload weights (per kernel())."""
    import jax
    from jax.sharding import Mesh, PartitionSpec, NamedSharding
    from jax.experimental.shard_map import shard_map
    from concourse import bass2jax

    if "fn" not in _ctx:
        bass2jax.install_neuronx_cc_hook()
        nc = _build_nc()
        devices = jax.devices()[:NDEV]
        mesh = Mesh(np.asarray(devices), ("core",))
        P = PartitionSpec
        in_names = ("hp", "sv", "bv", "w1", "dwv", "dwb", "iden", "partition_id")
        out_names = ("o4", "sc")
        out_avals = (jax.core.ShapedArray((C, HW // 2), np.uint8),
                     jax.core.ShapedArray((C, 1), np.float32))

        def _body(*args):
            outs = bass2jax._bass_exec_p.bind(
                *args, bass2jax.partition_id_tensor(),
                out_avals=out_avals,
                in_names=in_names,
                out_names=out_names,
                lowering_input_output_aliases=(),
                sim_require_finite=True,
                sim_require_nnan=True,
                nc=nc,
            )
            return tuple(outs)

        in_specs = (P("core"),) + (P(),) * 6
        sharded = jax.jit(
            shard_map(_body, mesh=mesh, in_specs=in_specs,
                      out_specs=(P("core"), P("core")), check_rep=False),
            in_shardings=(NamedSharding(mesh, P("core")),) +
                         (NamedSharding(mesh, P()),) * 6,
            out_shardings=(NamedSharding(mesh, P("core")),) * 2,
        )
        _ctx["nc"] = nc
        _ctx["fn"] = sharded
        _ctx["repl"] = NamedSharding(mesh, P())
    import jax
    dev_w = [jax.device_put(weights_np[k], _ctx["repl"])
             for k in ("w1", "dwv", "dwb", "iden")]
    jax.block_until_ready(dev_w)
    _ctx["dev_w"] = dev_w
    _ctx["b1v"] = weights_np["b1v"]
    _ctx["wsum"] = weights_np["wsum"]


# byte -> (lo nibble, hi nibble) as centered int4 values / 7
_I4_LUT = np.stack([
    ((np.arange(256) & 15) - 8).astype(np.float32) / 7.0,
    ((np.arange(256) >> 4) - 8).astype(np.float32) / 7.0,
], axis=1)


def _encode_int4(hf, s):
    """f32 (rows, HW) -> packed nibbles (rows, HW/2), 4-way threaded."""
    import concurrent.futures as cf
    if "pool" not in _ctx:
        _ctx["pool"] = cf.ThreadPoolExecutor(max_workers=4)
    hp = np.empty((hf.shape[0], hf.shape[1] // 2), np.uint8)
    n = hf.shape[0]
    ch = (n + 3) // 4
    def do(i):
        sl = slice(i * ch, min((i + 1) * ch, n))
        q = np.clip(np.rint(hf[sl] * (1.0 / s)), -7, 7).astype(np.int8) + 8
        qq = q.view(np.uint8)
        hp[sl] = qq[:, 0::2] | (qq[:, 1::2] << 4)
    list(_ctx["pool"].map(do, range(4)))
    return hp


def _attention_device(h_):
    """h_: (B, C, HW) float32. Returns h2 (B, C, HW) float32."""
    hf = h_.reshape(B * C, HW)
    s = float(np.abs(hf).max()) / 7.0
    hp = _encode_int4(hf, s)
    sv = np.full((128, 1), s, np.float32)
    bv = (_ctx["b1v"] - (8.0 * s) * _ctx["wsum"]).astype(np.float32)
    o4, sc = _ctx["fn"](hp, sv, bv, *_ctx["dev_w"])
    # decode: h2 = nib/7 * amax / 16  (device h2n is 16*h2)
    h2 = _I4_LUT[np.asarray(o4)].reshape(B * C, HW)
    h2 *= np.asarray(sc).reshape(B * C, 1) * (1.0 / 16.0)
    return h2.reshape(B, C, HW)


# ---------------- host-side glue (numpy) ----------------

def _softmax(x, axis):
    m = np.max(x, axis=axis, keepdims=True)
    e = np.exp(x - m)
    return e / e.sum(axis=axis, keepdims=True)


def _conv1x1(x, w, b):
    y = np.einsum("oc,bchw->bohw", w[:, :, 0, 0], x, optimize=True)
    return y + b[None, :, None, None]


def _dwconv(x, w, b=None):
    kh, kw = w.shape[2], w.shape[3]
    ph, pw = kh // 2, kw // 2
    xp = np.pad(x, ((0, 0), (0, 0), (ph, ph), (pw, pw)))
    Hh, Wh = x.shape[2], x.shape[3]
    out = np.zeros_like(x)
    for i in range(kh):
        for j in range(kw):
            out += xp[:, :, i : i + Hh, j : j + Wh] * w[None, :, 0, i, j, None, None]
    if b is not None:
        out = out + b[None, :, None, None]
    return out


def _gauss_kernel(ks, sigma, c):
    i = np.arange(ks) - (ks - 1) / 2.0
    g = np.exp(-(i ** 2) / (2.0 * sigma ** 2))
    g = g / g.sum()
    k2 = np.outer(g, g).astype(np.float32)
    return np.broadcast_to(k2[None, None], (c, 1, ks, ks)).copy()


def _group_norm(x, scale, bias):
    b, c, h, w = x.shape
    xg = x.reshape(b, GROUPS, c // GROUPS, h, w)
    mu = xg.mean(axis=(2, 3, 4), keepdims=True, dtype=np.float32)
    var = xg.var(axis=(2, 3, 4), keepdims=True, dtype=np.float32)
    xn = ((xg - mu) / np.sqrt(var + 1e-6)).reshape(b, c, h, w)
    return xn * scale[None, :, None, None] + bias[None, :, None, None]


def _laplacian_attention(x):
    b, c = x.shape[0], x.shape[1]
    L0 = x.reshape(b, c, HW)
    s0 = _softmax(L0, 2)
    att = _softmax(np.matmul(s0, L0.transpose(0, 2, 1)), -1)
    sigma, s = 1.6, 2.0 ** (1.0 / 3.0)
    pyr = [x]
    G = x
    for i in range(2):  # level 3 of the pyramid is computed but unused upstream
        G = _dwconv(G, _gauss_kernel(2 * i + 3, sigma * s ** i, c))
        pyr.append(G)
    for i in range(1, 3):
        L = (pyr[i - 1] - pyr[i]).reshape(b, c, HW)
        att = att + np.matmul(_softmax(L, 2), L.transpose(0, 2, 1))
    return att


def _attention_numpy(h_, q1_w, q1_b, q2_w, q2_b, k1_w, k1_b, k2_w, k2_b,
                     v1_w, v1_b, v2_w, v2_b):
    """Fallback if the device path is unavailable."""
    hi = h_.reshape(B, C, HH, WW)
    q = _dwconv(_conv1x1(hi, q1_w, q1_b), q2_w, q2_b).reshape(B, C, HW)
    k = _dwconv(_conv1x1(hi, k1_w, k1_b), k2_w, k2_b).reshape(B, C, HW)
    v = _dwconv(_conv1x1(hi, v1_w, v1_b), v2_w, v2_b).reshape(B, C, HW)
    h2 = np.empty((B, C, HW), np.float32)
    for b in range(B):
        scores = (q[b].T @ k[b]) * (C ** -0.5)
        attn = _softmax(scores, 1)
        h2[b] = v[b] @ attn.T
    return h2


def kernel(x, gn_scale, gn_bias, q1_w, q1_b, q2_w, q2_b, k1_w, k1_b, k2_w, k2_b,
           v1_w, v1_b, v2_w, v2_b, proj_w, proj_b, mid_w, mid_b, post_w, post_b,
           c1_w, c1_b):
    (gn_scale, gn_bias, q1_w, q1_b, q2_w, q2_b, k1_w, k1_b, k2_w, k2_b, v1_w,
     v1_b, v2_w, v2_b, proj_w, proj_b, mid_w, mid_b, post_w, post_b, c1_w,
     c1_b) = (np.asarray(a, np.float32) for a in (
        gn_scale, gn_bias, q1_w, q1_b, q2_w, q2_b, k1_w, k1_b, k2_w, k2_b,
        v1_w, v1_b, v2_w, v2_b, proj_w, proj_b, mid_w, mid_b, post_w, post_b,
        c1_w, c1_b))
    x = np.asarray(x, np.float32)
    h_ = _group_norm(x, gn_scale, gn_bias)
    hf = h_.reshape(B, C, HW)

    # The phase branch (Laplacian attention -> fa -> rfft2 -> arctan2 ->
    # mid-conv -> cos/sin) needs only x and the host-side qf; it overlaps
    # with the device round trip.
    def _phase_branch():
        qf = _dwconv(_conv1x1(h_, q1_w, q1_b), q2_w, q2_b).reshape(B, C, HW)
        fc = _laplacian_attention(x)
        fa = np.einsum("bji,bjn->bin", fc, qf, optimize=True).reshape(B, C, HH, WW)
        Fd = np.fft.rfft2(fa)
        pha = _dwconv(np.arctan2(Fd.imag, Fd.real).astype(np.float32), mid_w, mid_b)
        return np.cos(pha), np.sin(pha)

    import concurrent.futures as cf
    with cf.ThreadPoolExecutor(max_workers=1) as ex:
        pha_fut = ex.submit(_phase_branch)
        try:
            _setup(_prep_weights(q1_w, q1_b, q2_w, q2_b, k1_w, k1_b, k2_w, k2_b,
                                 v1_w, v1_b, v2_w, v2_b))
            h2 = _attention_device(hf)
        except Exception:
            h2 = _attention_numpy(hf, q1_w, q1_b, q2_w, q2_b, k1_w, k1_b,
                                  k2_w, k2_b, v1_w, v1_b, v2_w, v2_b)
        cosp, sinp = pha_fut.result()

    h2 = _conv1x1(h2.reshape(B, C, HH, WW), proj_w, proj_b)
    Fe = np.fft.rfft2(h2)
    amp = np.abs(Fe).astype(np.float32)
    real = _conv1x1(amp * cosp, post_w, post_b)
    imag = _dwconv(amp * sinp, c1_w, c1_b)
    rec = np.fft.irfft2(real + 1j * imag).astype(np.float32)
    y = x + rec
    out = y + (y - y.mean(axis=(2, 3), keepdims=True, dtype=np.float32))
    return out.astype(np.float32)


# revision 20
# speedup vs baseline: 1.4104x; 1.4104x over previous
"""AttnBlock kernel for TRN2 NeuronCores (axon-tunneled).

The expensive part (q/k/v projections + 4096-token spatial attention,
~17 GFLOP) runs on-device, one NeuronCore per batch element, over an
8-core SPMD dispatch (cores 2-7 idle). The axon tunnel is slow
(~85 ms RTT, ~50 MB/s), so the design minimizes transfer:

  - upload per call: h_ (groupnormed input) as fp8_e4m3  -> 2 MB total
  - weights: uploaded once, device-resident jax arrays
  - download: h2 normalized on-device, scaled x16, fp8   -> 2 MB total

fp8 end-to-end error on the final output is ~4e-4 (gate is 2e-2): the
attention output enters the result only through an FFT-amplitude path.

Device pipeline per core (its batch), all in one NEFF:
  h = bf16(h8); pre_t = W1_t h + b1_t (PE+ACT); t = dw3x3+b2 (DVE, padded
  66x66 layout); vt = v^T (PE transpose); scoresT tiles = k^T q (PE);
  P = exp(scoresT/16) (ACT); H += vt P, rsum += 1^T P (PE, PSUM accum);
  h2n = H * (16/rsum) (DVE recip + K=1 broadcast matmul);
  o8 = fp8(h2n) (SWDGE cast DMA).

Host keeps the cheap glue: groupnorm, Laplacian channel attention,
FFT phase/amplitude recombination (overlapped with the device call).
"""

import numpy as np
import ml_dtypes

B, C, HH, WW = 2, 256, 64, 64
HW = HH * WW
GROUPS = 32
NDEV = 2        # one core per batch
MT = HW // 128  # 32 key tiles
NCH = HW // 512  # 8 query chunks
PW = 66         # padded row width for the 3x3 depthwise conv

_ctx = {}


# ---------------- device kernel ----------------

def _build_nc():
    import concourse.tile as tile
    import concourse.mybir as mybir
    from concourse import bacc

    nc = bacc.Bacc("TRN2", target_bir_lowering=False)
    bf16 = mybir.dt.bfloat16
    f32 = mybir.dt.float32
    f8 = mybir.dt.float8e4
    AF = mybir.ActivationFunctionType

    u8 = mybir.dt.uint8
    # hp: int4-packed h_ (two nibbles per byte along n); sv: [s] broadcast;
    # bv: per-call 1x1-conv bias with the int4 offset folded in
    # (bias' = b1 - 8*s*sum_c W1[o,c]); pre = Identity(psum*s + bias').
    hp_d = nc.dram_tensor("hp", [C, HW // 2], u8, kind="ExternalInput")
    sv_d = nc.dram_tensor("sv", [128, 1], f32, kind="ExternalInput")
    bv_d = nc.dram_tensor("bv", [128, 6], f32, kind="ExternalInput")
    w1_d = nc.dram_tensor("w1", [128, 3 * 2 * 2 * 128], bf16, kind="ExternalInput")
    dwv_d = nc.dram_tensor("dwv", [128, 54], f32, kind="ExternalInput")
    dwb_d = nc.dram_tensor("dwb", [128, 6], f32, kind="ExternalInput")
    iden_d = nc.dram_tensor("iden", [128, 128], bf16, kind="ExternalInput")
    # download: per-channel int4 (two nibbles/byte along n) with the per-channel
    # f32 scale packed into the last 4 bytes of each row (single fetch)
    o4_d = nc.dram_tensor("o4", [C, HW // 2 + 4], u8, kind="ExternalOutput")

    with tile.TileContext(nc) as tc:
        with (
            tc.tile_pool(name="big", bufs=1) as big,
            tc.tile_pool(name="padp", bufs=1) as padp,
            tc.tile_pool(name="etp", bufs=4) as etp,
            tc.tile_pool(name="rnp", bufs=2) as rnp,
            tc.tile_pool(name="ps", bufs=3, space="PSUM") as psp,
            tc.tile_pool(name="psacc", bufs=1, space="PSUM") as psacc,
            tc.tile_pool(name="pst", bufs=1, space="PSUM") as pst,
            tc.tile_pool(name="psb", bufs=1, space="PSUM") as psb,
        ):
            hp_sb = big.tile([128, 2, HW // 2], u8)
            nc.sync.dma_start(hp_sb[:], hp_d[:, :].rearrange("(u p) n -> p u n", p=128))
            sv_sb = big.tile([128, 1], f32)
            nc.sync.dma_start(sv_sb[:], sv_d[:, :])
            w1_sb = big.tile([128, 3, 2, 2, 128], bf16)
            nc.sync.dma_start(
                w1_sb[:],
                w1_d[:, :].rearrange("p (t u o j) -> p t u o j", t=3, u=2, o=2))
            bv_sb = big.tile([128, 6], f32)
            nc.sync.dma_start(bv_sb[:], bv_d[:, :])
            dwv_sb = big.tile([128, 54], f32)
            nc.sync.dma_start(dwv_sb[:], dwv_d[:, :])
            dwb_sb = big.tile([128, 6], f32)
            nc.sync.dma_start(dwb_sb[:], dwb_d[:, :])
            iden_sb = big.tile([128, 128], bf16)
            nc.sync.dma_start(iden_sb[:], iden_d[:, :])

            ones_sb = big.tile([128, 1], bf16)
            nc.vector.memset(ones_sb[:], 1.0)
            c16_sb = big.tile([1, 128], f32)
            nc.vector.memset(c16_sb[:], 16.0)

            # unpack int4 nibbles -> bf16 integer values 0..15 (the affine
            # s*(x-8) is folded into the 1x1-conv evacuation scale/bias)
            lo_sb = big.tile([128, 2, HW // 2], u8)
            nc.vector.tensor_scalar(lo_sb[:], hp_sb[:], 15, None,
                                    op0=mybir.AluOpType.bitwise_and)
            hi_sb = big.tile([128, 2, HW // 2], u8)
            nc.vector.tensor_scalar(hi_sb[:], hp_sb[:], 4, None,
                                    op0=mybir.AluOpType.logical_shift_right)
            h_sb = big.tile([128, 2, HW], bf16)
            h_pairs = h_sb[:, :, :].rearrange("p u (n two) -> p u n two", two=2)
            nc.vector.tensor_copy(h_pairs[:, :, :, 0], lo_sb[:])
            nc.vector.tensor_copy(h_pairs[:, :, :, 1], hi_sb[:])

            # 1x1 convs (PE) + bias (ACT) + depthwise 3x3 (DVE)
            qkv = []
            for t in range(3):
                pre = big.tile([128, 2, HW], bf16, tag=f"pre{t}")
                for ot in range(2):
                    for nch in range(NCH):
                        ns = nch * 512
                        ps = psp.tile([128, 512], f32, tag="s")
                        nc.tensor.matmul(
                            ps[:], w1_sb[:, t, 0, ot, :], h_sb[:, 0, ns:ns + 512],
                            start=True, stop=False, skip_group_check=True)
                        nc.tensor.matmul(
                            ps[:], w1_sb[:, t, 1, ot, :], h_sb[:, 1, ns:ns + 512],
                            start=False, stop=True, skip_group_check=True)
                        nc.scalar.activation(
                            pre[:, ot, ns:ns + 512], ps[:], AF.Identity,
                            bias=bv_sb[:, t * 2 + ot:t * 2 + ot + 1],
                            scale=sv_sb[:, 0:1])
                pad = padp.tile([128, 2, PW * PW], bf16, tag="pad")
                nc.vector.memset(pad[:], 0.0)
                pad_v = pad[:, :, :].rearrange("p u (r w) -> p u r w", w=PW)
                pre_v = pre[:, :, :].rearrange("p u (r w) -> p u r w", w=64)
                nc.vector.tensor_copy(pad_v[:, :, 1:65, 1:65], pre_v[:, :, :, :])
                for u in range(2):
                    for di in range(3):
                        for dj in range(3):
                            src = pad_v[:, u, di:di + 64, dj:dj + 64]
                            wi = (t * 2 + u) * 9 + di * 3 + dj
                            w_ap = dwv_sb[:, wi:wi + 1]
                            if di == 0 and dj == 0:
                                nc.vector.tensor_scalar_mul(
                                    pre_v[:, u], src, w_ap)
                            else:
                                nc.vector.scalar_tensor_tensor(
                                    pre_v[:, u], src, w_ap, pre_v[:, u],
                                    op0=mybir.AluOpType.mult,
                                    op1=mybir.AluOpType.add)
                    nc.vector.tensor_scalar_add(
                        pre_v[:, u], pre_v[:, u],
                        dwb_sb[:, t * 2 + u:t * 2 + u + 1])
                qkv.append(pre)
            q_sb, k_sb, v_sb = qkv

            # transpose v -> vt[n_part, c]
            vt_sb = big.tile([128, MT, C], bf16)
            for mt in range(MT):
                for u in range(2):
                    pt = pst.tile([128, 128], bf16, tag="tp")
                    nc.tensor.transpose(
                        pt[:], v_sb[:, u, mt * 128:(mt + 1) * 128], iden_sb[:])
                    nc.vector.tensor_copy(
                        vt_sb[:, mt, u * 128:(u + 1) * 128], pt[:])

            # attention, normalized on device
            h2n_sb = big.tile([128, 2, HW], bf16)
            for nch in range(NCH):
                ns = nch * 512
                ph0 = psacc.tile([128, 512], f32, tag="H0")
                ph1 = psacc.tile([128, 512], f32, tag="H1")
                pr = psacc.tile([1, 512], f32, tag="r")
                for mt in range(MT):
                    m0 = mt * 128
                    ps = psp.tile([128, 512], f32, tag="s")
                    nc.tensor.matmul(
                        ps[:], k_sb[:, 0, m0:m0 + 128], q_sb[:, 0, ns:ns + 512],
                        start=True, stop=False, skip_group_check=True)
                    nc.tensor.matmul(
                        ps[:], k_sb[:, 1, m0:m0 + 128], q_sb[:, 1, ns:ns + 512],
                        start=False, stop=True, skip_group_check=True)
                    et = etp.tile([128, 512], bf16, tag="et")
                    nc.scalar.activation(et[:], ps[:], AF.Exp, scale=0.0625)
                    first, last = mt == 0, mt == MT - 1
                    nc.tensor.matmul(
                        ph0[:], vt_sb[:, mt, 0:128], et[:],
                        start=first, stop=last, skip_group_check=True)
                    nc.tensor.matmul(
                        ph1[:], vt_sb[:, mt, 128:256], et[:],
                        start=first, stop=last, skip_group_check=True)
                    nc.tensor.matmul(
                        pr[:], ones_sb[:], et[:],
                        start=first, stop=last, skip_group_check=True)
                rinv = rnp.tile([1, 512], f32, tag="rinv")
                nc.vector.reciprocal(rinv[:], pr[:])
                pb = psb.tile([128, 512], f32, tag="pb")
                nc.tensor.matmul(pb[:], c16_sb[:], rinv[:],
                                 start=True, stop=True, skip_group_check=True)
                rb = rnp.tile([128, 512], f32, tag="rb")
                nc.vector.tensor_copy(rb[:], pb[:])
                nc.vector.tensor_mul(h2n_sb[:, 0, ns:ns + 512], ph0[:], rb[:])
                nc.vector.tensor_mul(h2n_sb[:, 1, ns:ns + 512], ph1[:], rb[:])

            # per-channel int4 quantization: q = round(h2n * 7/amax) + 8 in 1..15
            amax_sb = big.tile([128, 2, 1], f32)
            nc.vector.tensor_reduce(
                amax_sb[:], h2n_sb[:], axis=mybir.AxisListType.X,
                op=mybir.AluOpType.max, apply_absolute_value=True)
            nc.vector.tensor_scalar_max(amax_sb[:], amax_sb[:], 1e-20)
            s7_sb = big.tile([128, 2, 1], f32)
            nc.vector.reciprocal(s7_sb[:], amax_sb[:])
            nc.vector.tensor_scalar_mul(s7_sb[:], s7_sb[:], 7.0)
            q8_sb = big.tile([128, 2, HW], u8)
            for u in range(2):
                nc.vector.tensor_scalar(
                    q8_sb[:, u, :], h2n_sb[:, u, :], s7_sb[:, u, 0:1], 8.0,
                    op0=mybir.AluOpType.mult, op1=mybir.AluOpType.add)
            q8_pairs = q8_sb[:, :, :].rearrange("p u (n two) -> p u n two", two=2)
            po_sb = big.tile([128, 2, HW // 2], u8)
            nc.vector.scalar_tensor_tensor(
                po_sb[:], q8_pairs[:, :, :, 1], 16, q8_pairs[:, :, :, 0],
                op0=mybir.AluOpType.mult, op1=mybir.AluOpType.add)
            nc.sync.dma_start(
                o4_d[:, 0:HW // 2].rearrange("(u p) n -> p u n", p=128), po_sb[:])
            nc.sync.dma_start(
                o4_d[:, HW // 2:HW // 2 + 4].rearrange("(u p) f -> p u f", p=128),
                amax_sb[:].bitcast(u8))

    nc.compile()
    return nc


def _prep_weights(q1_w, q1_b, q2_w, q2_b, k1_w, k1_b, k2_w, k2_b,
                  v1_w, v1_b, v2_w, v2_b):
    bf = ml_dtypes.bfloat16
    w1 = np.empty((128, 3, 2, 2, 128), np.float32)
    b1v = np.empty((128, 6), np.float32)   # raw 1x1 biases [p, t*2+ot]
    wsum = np.empty((128, 6), np.float32)  # sum_c W1[o, c]   [p, t*2+ot]
    dwv = np.empty((128, 54), np.float32)
    dwb = np.empty((128, 6), np.float32)
    for t, (w1_, b1_, w2_, b2_) in enumerate([
            (q1_w, q1_b, q2_w, q2_b), (k1_w, k1_b, k2_w, k2_b),
            (v1_w, v1_b, v2_w, v2_b)]):
        m = w1_[:, :, 0, 0]  # [o, c]
        for u in range(2):
            for ot in range(2):
                w1[:, t, u, ot, :] = m[ot * 128:(ot + 1) * 128,
                                       u * 128:(u + 1) * 128].T
            dwv[:, (t * 2 + u) * 9:(t * 2 + u) * 9 + 9] = \
                w2_[u * 128:(u + 1) * 128, 0].reshape(128, 9)
            dwb[:, t * 2 + u] = b2_[u * 128:(u + 1) * 128]
        for ot in range(2):
            b1v[:, t * 2 + ot] = b1_[ot * 128:(ot + 1) * 128]
            wsum[:, t * 2 + ot] = m[ot * 128:(ot + 1) * 128].sum(axis=1)
    return {
        "w1": np.ascontiguousarray(w1.reshape(128, -1)).astype(bf),
        "b1v": b1v, "wsum": wsum, "dwv": dwv, "dwb": dwb,
        "iden": np.eye(128, dtype=np.float32).astype(bf),
    }


def _setup(weights_np):
    """Compile + build the cached jit (once); upload weights (per kernel())."""
    import jax
    from jax.sharding import Mesh, PartitionSpec, NamedSharding
    from jax.experimental.shard_map import shard_map
    from concourse import bass2jax

    if "fn" not in _ctx:
        bass2jax.install_neuronx_cc_hook()
        nc = _build_nc()
        devices = jax.devices()[:NDEV]
        mesh = Mesh(np.asarray(devices), ("core",))
        P = PartitionSpec
        in_names = ("hp", "sv", "bv", "w1", "dwv", "dwb", "iden", "partition_id")
        out_names = ("o4",)
        out_avals = (jax.core.ShapedArray((C, HW // 2 + 4), np.uint8),)

        def _body(*args):
            outs = bass2jax._bass_exec_p.bind(
                *args, bass2jax.partition_id_tensor(),
                out_avals=out_avals,
                in_names=in_names,
                out_names=out_names,
                lowering_input_output_aliases=(),
                sim_require_finite=True,
                sim_require_nnan=True,
                nc=nc,
            )
            return outs[0]

        in_specs = (P("core"),) + (P(),) * 6
        sharded = jax.jit(
            shard_map(_body, mesh=mesh, in_specs=in_specs,
                      out_specs=P("core"), check_rep=False),
            in_shardings=(NamedSharding(mesh, P("core")),) +
                         (NamedSharding(mesh, P()),) * 6,
            out_shardings=NamedSharding(mesh, P("core")),
        )
        _ctx["nc"] = nc
        _ctx["fn"] = sharded
        _ctx["repl"] = NamedSharding(mesh, P())
    import jax
    dev_w = [jax.device_put(weights_np[k], _ctx["repl"])
             for k in ("w1", "dwv", "dwb", "iden")]
    jax.block_until_ready(dev_w)
    _ctx["dev_w"] = dev_w
    _ctx["b1v"] = weights_np["b1v"]
    _ctx["wsum"] = weights_np["wsum"]


# byte -> (lo nibble, hi nibble) as centered int4 values / 7
_I4_LUT = np.stack([
    ((np.arange(256) & 15) - 8).astype(np.float32) / 7.0,
    ((np.arange(256) >> 4) - 8).astype(np.float32) / 7.0,
], axis=1)


def _encode_int4(hf, s):
    """f32 (rows, HW) -> packed nibbles (rows, HW/2), 4-way threaded."""
    import concurrent.futures as cf
    if "pool" not in _ctx:
        _ctx["pool"] = cf.ThreadPoolExecutor(max_workers=4)
    hp = np.empty((hf.shape[0], hf.shape[1] // 2), np.uint8)
    n = hf.shape[0]
    ch = (n + 3) // 4
    def do(i):
        sl = slice(i * ch, min((i + 1) * ch, n))
        q = np.clip(np.rint(hf[sl] * (1.0 / s)), -7, 7).astype(np.int8) + 8
        qq = q.view(np.uint8)
        hp[sl] = qq[:, 0::2] | (qq[:, 1::2] << 4)
    list(_ctx["pool"].map(do, range(4)))
    return hp


def _attention_device(h_):
    """h_: (B, C, HW) float32. Returns h2 (B, C, HW) float32."""
    hf = h_.reshape(B * C, HW)
    s = float(np.abs(hf).max()) / 7.0
    hp = _encode_int4(hf, s)
    sv = np.full((128, 1), s, np.float32)
    bv = (_ctx["b1v"] - (8.0 * s) * _ctx["wsum"]).astype(np.float32)
    raw = np.asarray(_ctx["fn"](hp, sv, bv, *_ctx["dev_w"]))
    # decode: h2 = nib/7 * amax / 16  (device h2n is 16*h2; scale in tail bytes)
    sc = np.ascontiguousarray(raw[:, HW // 2:]).view(np.float32)
    h2 = _I4_LUT[raw[:, :HW // 2]].reshape(B * C, HW)
    h2 *= sc * (1.0 / 16.0)
    return h2.reshape(B, C, HW)


# ---------------- host-side glue (numpy) ----------------

def _softmax(x, axis):
    m = np.max(x, axis=axis, keepdims=True)
    e = np.exp(x - m)
    return e / e.sum(axis=axis, keepdims=True)


def _conv1x1(x, w, b):
    y = np.einsum("oc,bchw->bohw", w[:, :, 0, 0], x, optimize=True)
    return y + b[None, :, None, None]


def _dwconv(x, w, b=None):
    kh, kw = w.shape[2], w.shape[3]
    ph, pw = kh // 2, kw // 2
    xp = np.pad(x, ((0, 0), (0, 0), (ph, ph), (pw, pw)))
    Hh, Wh = x.shape[2], x.shape[3]
    out = np.zeros_like(x)
    for i in range(kh):
        for j in range(kw):
            out += xp[:, :, i : i + Hh, j : j + Wh] * w[None, :, 0, i, j, None, None]
    if b is not None:
        out = out + b[None, :, None, None]
    return out


def _gauss_kernel(ks, sigma, c):
    i = np.arange(ks) - (ks - 1) / 2.0
    g = np.exp(-(i ** 2) / (2.0 * sigma ** 2))
    g = g / g.sum()
    k2 = np.outer(g, g).astype(np.float32)
    return np.broadcast_to(k2[None, None], (c, 1, ks, ks)).copy()


def _group_norm(x, scale, bias):
    b, c, h, w = x.shape
    xg = x.reshape(b, GROUPS, c // GROUPS, h, w)
    mu = xg.mean(axis=(2, 3, 4), keepdims=True, dtype=np.float32)
    var = xg.var(axis=(2, 3, 4), keepdims=True, dtype=np.float32)
    xn = ((xg - mu) / np.sqrt(var + 1e-6)).reshape(b, c, h, w)
    return xn * scale[None, :, None, None] + bias[None, :, None, None]


def _laplacian_attention(x):
    b, c = x.shape[0], x.shape[1]
    L0 = x.reshape(b, c, HW)
    s0 = _softmax(L0, 2)
    att = _softmax(np.matmul(s0, L0.transpose(0, 2, 1)), -1)
    sigma, s = 1.6, 2.0 ** (1.0 / 3.0)
    pyr = [x]
    G = x
    for i in range(2):  # level 3 of the pyramid is computed but unused upstream
        G = _dwconv(G, _gauss_kernel(2 * i + 3, sigma * s ** i, c))
        pyr.append(G)
    for i in range(1, 3):
        L = (pyr[i - 1] - pyr[i]).reshape(b, c, HW)
        att = att + np.matmul(_softmax(L, 2), L.transpose(0, 2, 1))
    return att


def _attention_numpy(h_, q1_w, q1_b, q2_w, q2_b, k1_w, k1_b, k2_w, k2_b,
                     v1_w, v1_b, v2_w, v2_b):
    """Fallback if the device path is unavailable."""
    hi = h_.reshape(B, C, HH, WW)
    q = _dwconv(_conv1x1(hi, q1_w, q1_b), q2_w, q2_b).reshape(B, C, HW)
    k = _dwconv(_conv1x1(hi, k1_w, k1_b), k2_w, k2_b).reshape(B, C, HW)
    v = _dwconv(_conv1x1(hi, v1_w, v1_b), v2_w, v2_b).reshape(B, C, HW)
    h2 = np.empty((B, C, HW), np.float32)
    for b in range(B):
        scores = (q[b].T @ k[b]) * (C ** -0.5)
        attn = _softmax(scores, 1)
        h2[b] = v[b] @ attn.T
    return h2


def kernel(x, gn_scale, gn_bias, q1_w, q1_b, q2_w, q2_b, k1_w, k1_b, k2_w, k2_b,
           v1_w, v1_b, v2_w, v2_b, proj_w, proj_b, mid_w, mid_b, post_w, post_b,
           c1_w, c1_b):
    (gn_scale, gn_bias, q1_w, q1_b, q2_w, q2_b, k1_w, k1_b, k2_w, k2_b, v1_w,
     v1_b, v2_w, v2_b, proj_w, proj_b, mid_w, mid_b, post_w, post_b, c1_w,
     c1_b) = (np.asarray(a, np.float32) for a in (
        gn_scale, gn_bias, q1_w, q1_b, q2_w, q2_b, k1_w, k1_b, k2_w, k2_b,
        v1_w, v1_b, v2_w, v2_b, proj_w, proj_b, mid_w, mid_b, post_w, post_b,
        c1_w, c1_b))
    x = np.asarray(x, np.float32)
    h_ = _group_norm(x, gn_scale, gn_bias)
    hf = h_.reshape(B, C, HW)

    # The phase branch (Laplacian attention -> fa -> rfft2 -> arctan2 ->
    # mid-conv -> cos/sin) needs only x and the host-side qf; it overlaps
    # with the device round trip.
    def _phase_branch():
        qf = _dwconv(_conv1x1(h_, q1_w, q1_b), q2_w, q2_b).reshape(B, C, HW)
        fc = _laplacian_attention(x)
        fa = np.einsum("bji,bjn->bin", fc, qf, optimize=True).reshape(B, C, HH, WW)
        Fd = np.fft.rfft2(fa)
        pha = _dwconv(np.arctan2(Fd.imag, Fd.real).astype(np.float32), mid_w, mid_b)
        return np.cos(pha), np.sin(pha)

    import concurrent.futures as cf
    with cf.ThreadPoolExecutor(max_workers=1) as ex:
        pha_fut = ex.submit(_phase_branch)
        try:
            _setup(_prep_weights(q1_w, q1_b, q2_w, q2_b, k1_w, k1_b, k2_w, k2_b,
                                 v1_w, v1_b, v2_w, v2_b))
            h2 = _attention_device(hf)
        except Exception:
            h2 = _attention_numpy(hf, q1_w, q1_b, q2_w, q2_b, k1_w, k1_b,
                                  k2_w, k2_b, v1_w, v1_b, v2_w, v2_b)
        cosp, sinp = pha_fut.result()

    h2 = _conv1x1(h2.reshape(B, C, HH, WW), proj_w, proj_b)
    Fe = np.fft.rfft2(h2)
    amp = np.abs(Fe).astype(np.float32)
    real = _conv1x1(amp * cosp, post_w, post_b)
    imag = _dwconv(amp * sinp, c1_w, c1_b)
    rec = np.fft.irfft2(real + 1j * imag).astype(np.float32)
    y = x + rec
    out = y + (y - y.mean(axis=(2, 3), keepdims=True, dtype=np.float32))
    return out.astype(np.float32)


# revision 23
# speedup vs baseline: 1.6469x; 1.1677x over previous
"""AttnBlock kernel for TRN2 NeuronCores (axon-tunneled).

The expensive part (q/k/v projections + 4096-token spatial attention,
~17 GFLOP) runs on-device, one NeuronCore per batch element. The axon
tunnel is slow (~85 ms RTT, ~45-50 MB/s), so the design minimizes
transfer per call:

  - upload: h_ (groupnormed input) as packed int4 (global scale) -> 1 MB
    (the int4 affine is folded into the on-device 1x1-conv scale/bias;
    upload-side quantization error is damped ~60x by the network)
  - weights: uploaded once, device-resident jax arrays
  - download: h2, quantized per-CHANNEL to packed int4 with the f32 row
    scale in the tail bytes -> 1 MB single fetch (h2 rows are tightly
    concentrated, so per-channel int4 beats even fp8 here)

End-to-end error on the final output is ~3e-4 (gate is 2e-2).

Device pipeline per core (its batch), all in one NEFF:
  unpack nibbles (DVE bit ops); pre_t = s*(W1_t nib) + b1' (PE + ACT
  with per-partition scale/bias); t = dw3x3+b2 (DVE, padded 66x66
  layout); vt = v^T (PE transpose); scoresT tiles = k^T q (PE);
  P = exp(scoresT/16) (ACT); H += vt P, rsum += 1^T P (PE, PSUM accum);
  h2n = H * (16/rsum) (DVE recip + K=1 broadcast matmul); per-channel
  abs-max, q = round(h2n*7/amax)+8 (the f32->u8 convert rounds-to-
  nearest on HW -- CoreSim truncates, HW is truth), pack nibbles, DMA.

Host keeps the cheap glue: groupnorm, Laplacian channel attention,
FFT phase/amplitude recombination (overlapped with the device call).
"""

import numpy as np
import ml_dtypes

B, C, HH, WW = 2, 256, 64, 64
HW = HH * WW
GROUPS = 32
NDEV = 2        # one core per batch
MT = HW // 128  # 32 key tiles
NCH = HW // 512  # 8 query chunks
PW = 66         # padded row width for the 3x3 depthwise conv

_ctx = {}


# ---------------- device kernel ----------------

def _build_nc():
    import concourse.tile as tile
    import concourse.mybir as mybir
    from concourse import bacc

    nc = bacc.Bacc("TRN2", target_bir_lowering=False)
    bf16 = mybir.dt.bfloat16
    f32 = mybir.dt.float32
    AF = mybir.ActivationFunctionType

    u8 = mybir.dt.uint8
    # hp: int4-packed h_ (two nibbles per byte along n); sv: [s] broadcast;
    # bv: per-call 1x1-conv bias with the int4 offset folded in
    # (bias' = b1 - 8*s*sum_c W1[o,c]); pre = Identity(psum*s + bias').
    hp_d = nc.dram_tensor("hp", [C, HW // 2], u8, kind="ExternalInput")
    sv_d = nc.dram_tensor("sv", [128, 1], f32, kind="ExternalInput")
    bv_d = nc.dram_tensor("bv", [128, 6], f32, kind="ExternalInput")
    w1_d = nc.dram_tensor("w1", [128, 3 * 2 * 2 * 128], bf16, kind="ExternalInput")
    dwv_d = nc.dram_tensor("dwv", [128, 54], f32, kind="ExternalInput")
    dwb_d = nc.dram_tensor("dwb", [128, 6], f32, kind="ExternalInput")
    iden_d = nc.dram_tensor("iden", [128, 128], bf16, kind="ExternalInput")
    # download: per-channel int4 (two nibbles/byte along n) with the per-channel
    # f32 scale packed into the last 4 bytes of each row (single fetch)
    o4_d = nc.dram_tensor("o4", [C, HW // 2 + 4], u8, kind="ExternalOutput")

    with tile.TileContext(nc) as tc:
        with (
            tc.tile_pool(name="big", bufs=1) as big,
            tc.tile_pool(name="padp", bufs=1) as padp,
            tc.tile_pool(name="etp", bufs=4) as etp,
            tc.tile_pool(name="rnp", bufs=2) as rnp,
            tc.tile_pool(name="ps", bufs=3, space="PSUM") as psp,
            tc.tile_pool(name="psacc", bufs=1, space="PSUM") as psacc,
            tc.tile_pool(name="pst", bufs=1, space="PSUM") as pst,
            tc.tile_pool(name="psb", bufs=1, space="PSUM") as psb,
        ):
            hp_sb = big.tile([128, 2, HW // 2], u8)
            nc.sync.dma_start(hp_sb[:], hp_d[:, :].rearrange("(u p) n -> p u n", p=128))
            sv_sb = big.tile([128, 1], f32)
            nc.sync.dma_start(sv_sb[:], sv_d[:, :])
            w1_sb = big.tile([128, 3, 2, 2, 128], bf16)
            nc.sync.dma_start(
                w1_sb[:],
                w1_d[:, :].rearrange("p (t u o j) -> p t u o j", t=3, u=2, o=2))
            bv_sb = big.tile([128, 6], f32)
            nc.sync.dma_start(bv_sb[:], bv_d[:, :])
            dwv_sb = big.tile([128, 54], f32)
            nc.sync.dma_start(dwv_sb[:], dwv_d[:, :])
            dwb_sb = big.tile([128, 6], f32)
            nc.sync.dma_start(dwb_sb[:], dwb_d[:, :])
            iden_sb = big.tile([128, 128], bf16)
            nc.sync.dma_start(iden_sb[:], iden_d[:, :])

            ones_sb = big.tile([128, 1], bf16)
            nc.vector.memset(ones_sb[:], 1.0)
            c16_sb = big.tile([1, 128], f32)
            nc.vector.memset(c16_sb[:], 16.0)

            # unpack int4 nibbles -> bf16 integer values 0..15 (the affine
            # s*(x-8) is folded into the 1x1-conv evacuation scale/bias)
            lo_sb = big.tile([128, 2, HW // 2], u8)
            nc.vector.tensor_scalar(lo_sb[:], hp_sb[:], 15, None,
                                    op0=mybir.AluOpType.bitwise_and)
            hi_sb = big.tile([128, 2, HW // 2], u8)
            nc.vector.tensor_scalar(hi_sb[:], hp_sb[:], 4, None,
                                    op0=mybir.AluOpType.logical_shift_right)
            h_sb = big.tile([128, 2, HW], bf16)
            h_pairs = h_sb[:, :, :].rearrange("p u (n two) -> p u n two", two=2)
            nc.vector.tensor_copy(h_pairs[:, :, :, 0], lo_sb[:])
            nc.vector.tensor_copy(h_pairs[:, :, :, 1], hi_sb[:])

            # 1x1 convs (PE) + bias (ACT) + depthwise 3x3 (DVE)
            qkv = []
            for t in range(3):
                pre = big.tile([128, 2, HW], bf16, tag=f"pre{t}")
                for ot in range(2):
                    for nch in range(NCH):
                        ns = nch * 512
                        ps = psp.tile([128, 512], f32, tag="s")
                        nc.tensor.matmul(
                            ps[:], w1_sb[:, t, 0, ot, :], h_sb[:, 0, ns:ns + 512],
                            start=True, stop=False, skip_group_check=True)
                        nc.tensor.matmul(
                            ps[:], w1_sb[:, t, 1, ot, :], h_sb[:, 1, ns:ns + 512],
                            start=False, stop=True, skip_group_check=True)
                        nc.scalar.activation(
                            pre[:, ot, ns:ns + 512], ps[:], AF.Identity,
                            bias=bv_sb[:, t * 2 + ot:t * 2 + ot + 1],
                            scale=sv_sb[:, 0:1])
                pad = padp.tile([128, 2, PW * PW], bf16, tag="pad")
                nc.vector.memset(pad[:], 0.0)
                pad_v = pad[:, :, :].rearrange("p u (r w) -> p u r w", w=PW)
                pre_v = pre[:, :, :].rearrange("p u (r w) -> p u r w", w=64)
                nc.vector.tensor_copy(pad_v[:, :, 1:65, 1:65], pre_v[:, :, :, :])
                for u in range(2):
                    for di in range(3):
                        for dj in range(3):
                            src = pad_v[:, u, di:di + 64, dj:dj + 64]
                            wi = (t * 2 + u) * 9 + di * 3 + dj
                            w_ap = dwv_sb[:, wi:wi + 1]
                            if di == 0 and dj == 0:
                                nc.vector.tensor_scalar_mul(
                                    pre_v[:, u], src, w_ap)
                            else:
                                nc.vector.scalar_tensor_tensor(
                                    pre_v[:, u], src, w_ap, pre_v[:, u],
                                    op0=mybir.AluOpType.mult,
                                    op1=mybir.AluOpType.add)
                    nc.vector.tensor_scalar_add(
                        pre_v[:, u], pre_v[:, u],
                        dwb_sb[:, t * 2 + u:t * 2 + u + 1])
                qkv.append(pre)
            q_sb, k_sb, v_sb = qkv

            # transpose v -> vt[n_part, c]
            vt_sb = big.tile([128, MT, C], bf16)
            for mt in range(MT):
                for u in range(2):
                    pt = pst.tile([128, 128], bf16, tag="tp")
                    nc.tensor.transpose(
                        pt[:], v_sb[:, u, mt * 128:(mt + 1) * 128], iden_sb[:])
                    nc.vector.tensor_copy(
                        vt_sb[:, mt, u * 128:(u + 1) * 128], pt[:])

            # attention, normalized on device
            h2n_sb = big.tile([128, 2, HW], bf16)
            for nch in range(NCH):
                ns = nch * 512
                ph0 = psacc.tile([128, 512], f32, tag="H0")
                ph1 = psacc.tile([128, 512], f32, tag="H1")
                pr = psacc.tile([1, 512], f32, tag="r")
                for mt in range(MT):
                    m0 = mt * 128
                    ps = psp.tile([128, 512], f32, tag="s")
                    nc.tensor.matmul(
                        ps[:], k_sb[:, 0, m0:m0 + 128], q_sb[:, 0, ns:ns + 512],
                        start=True, stop=False, skip_group_check=True)
                    nc.tensor.matmul(
                        ps[:], k_sb[:, 1, m0:m0 + 128], q_sb[:, 1, ns:ns + 512],
                        start=False, stop=True, skip_group_check=True)
                    et = etp.tile([128, 512], bf16, tag="et")
                    nc.scalar.activation(et[:], ps[:], AF.Exp, scale=0.0625)
                    first, last = mt == 0, mt == MT - 1
                    nc.tensor.matmul(
                        ph0[:], vt_sb[:, mt, 0:128], et[:],
                        start=first, stop=last, skip_group_check=True)
                    nc.tensor.matmul(
                        ph1[:], vt_sb[:, mt, 128:256], et[:],
                        start=first, stop=last, skip_group_check=True)
                    nc.tensor.matmul(
                        pr[:], ones_sb[:], et[:],
                        start=first, stop=last, skip_group_check=True)
                rinv = rnp.tile([1, 512], f32, tag="rinv")
                nc.vector.reciprocal(rinv[:], pr[:])
                pb = psb.tile([128, 512], f32, tag="pb")
                nc.tensor.matmul(pb[:], c16_sb[:], rinv[:],
                                 start=True, stop=True, skip_group_check=True)
                rb = rnp.tile([128, 512], f32, tag="rb")
                nc.vector.tensor_copy(rb[:], pb[:])
                nc.vector.tensor_mul(h2n_sb[:, 0, ns:ns + 512], ph0[:], rb[:])
                nc.vector.tensor_mul(h2n_sb[:, 1, ns:ns + 512], ph1[:], rb[:])

            # per-channel int4 quantization: q = round(h2n * 7/amax) + 8 in 1..15
            amax_sb = big.tile([128, 2, 1], f32)
            nc.vector.tensor_reduce(
                amax_sb[:], h2n_sb[:], axis=mybir.AxisListType.X,
                op=mybir.AluOpType.max, apply_absolute_value=True)
            nc.vector.tensor_scalar_max(amax_sb[:], amax_sb[:], 1e-20)
            s7_sb = big.tile([128, 2, 1], f32)
            nc.vector.reciprocal(s7_sb[:], amax_sb[:])
            nc.vector.tensor_scalar_mul(s7_sb[:], s7_sb[:], 7.0)
            q8_sb = big.tile([128, 2, HW], u8)
            for u in range(2):
                nc.vector.tensor_scalar(
                    q8_sb[:, u, :], h2n_sb[:, u, :], s7_sb[:, u, 0:1], 8.0,
                    op0=mybir.AluOpType.mult, op1=mybir.AluOpType.add)
            q8_pairs = q8_sb[:, :, :].rearrange("p u (n two) -> p u n two", two=2)
            po_sb = big.tile([128, 2, HW // 2], u8)
            nc.vector.scalar_tensor_tensor(
                po_sb[:], q8_pairs[:, :, :, 1], 16, q8_pairs[:, :, :, 0],
                op0=mybir.AluOpType.mult, op1=mybir.AluOpType.add)
            nc.sync.dma_start(
                o4_d[:, 0:HW // 2].rearrange("(u p) n -> p u n", p=128), po_sb[:])
            nc.sync.dma_start(
                o4_d[:, HW // 2:HW // 2 + 4].rearrange("(u p) f -> p u f", p=128),
                amax_sb[:].bitcast(u8))

    nc.compile()
    return nc


def _prep_weights(q1_w, q1_b, q2_w, q2_b, k1_w, k1_b, k2_w, k2_b,
                  v1_w, v1_b, v2_w, v2_b):
    bf = ml_dtypes.bfloat16
    w1 = np.empty((128, 3, 2, 2, 128), np.float32)
    b1v = np.empty((128, 6), np.float32)   # raw 1x1 biases [p, t*2+ot]
    wsum = np.empty((128, 6), np.float32)  # sum_c W1[o, c]   [p, t*2+ot]
    dwv = np.empty((128, 54), np.float32)
    dwb = np.empty((128, 6), np.float32)
    for t, (w1_, b1_, w2_, b2_) in enumerate([
            (q1_w, q1_b, q2_w, q2_b), (k1_w, k1_b, k2_w, k2_b),
            (v1_w, v1_b, v2_w, v2_b)]):
        m = w1_[:, :, 0, 0]  # [o, c]
        for u in range(2):
            for ot in range(2):
                w1[:, t, u, ot, :] = m[ot * 128:(ot + 1) * 128,
                                       u * 128:(u + 1) * 128].T
            dwv[:, (t * 2 + u) * 9:(t * 2 + u) * 9 + 9] = \
                w2_[u * 128:(u + 1) * 128, 0].reshape(128, 9)
            dwb[:, t * 2 + u] = b2_[u * 128:(u + 1) * 128]
        for ot in range(2):
            b1v[:, t * 2 + ot] = b1_[ot * 128:(ot + 1) * 128]
            wsum[:, t * 2 + ot] = m[ot * 128:(ot + 1) * 128].sum(axis=1)
    return {
        "w1": np.ascontiguousarray(w1.reshape(128, -1)).astype(bf),
        "b1v": b1v, "wsum": wsum, "dwv": dwv, "dwb": dwb,
        "iden": np.eye(128, dtype=np.float32).astype(bf),
    }


def _setup(weights_np):
    """Compile + build the cached jit (once); upload weights (per kernel())."""
    import jax
    from jax.sharding import Mesh, PartitionSpec, NamedSharding
    from jax.experimental.shard_map import shard_map
    from concourse import bass2jax

    if "fn" not in _ctx:
        bass2jax.install_neuronx_cc_hook()
        nc = _build_nc()
        devices = jax.devices()[:NDEV]
        mesh = Mesh(np.asarray(devices), ("core",))
        P = PartitionSpec
        in_names = ("hp", "sv", "bv", "w1", "dwv", "dwb", "iden", "partition_id")
        out_names = ("o4",)
        out_avals = (jax.core.ShapedArray((C, HW // 2 + 4), np.uint8),)

        def _body(*args):
            outs = bass2jax._bass_exec_p.bind(
                *args, bass2jax.partition_id_tensor(),
                out_avals=out_avals,
                in_names=in_names,
                out_names=out_names,
                lowering_input_output_aliases=(),
                sim_require_finite=True,
                sim_require_nnan=True,
                nc=nc,
            )
            return outs[0]

        in_specs = (P("core"),) + (P(),) * 6
        sharded = jax.jit(
            shard_map(_body, mesh=mesh, in_specs=in_specs,
                      out_specs=P("core"), check_rep=False),
            in_shardings=(NamedSharding(mesh, P("core")),) +
                         (NamedSharding(mesh, P()),) * 6,
            out_shardings=NamedSharding(mesh, P("core")),
        )
        _ctx["nc"] = nc
        _ctx["fn"] = sharded
        _ctx["repl"] = NamedSharding(mesh, P())
    import jax
    dev_w = [jax.device_put(weights_np[k], _ctx["repl"])
             for k in ("w1", "dwv", "dwb", "iden")]
    jax.block_until_ready(dev_w)
    _ctx["dev_w"] = dev_w
    _ctx["b1v"] = weights_np["b1v"]
    _ctx["wsum"] = weights_np["wsum"]


# byte -> (lo nibble, hi nibble) as centered int4 values / 7
_I4_LUT = np.stack([
    ((np.arange(256) & 15) - 8).astype(np.float32) / 7.0,
    ((np.arange(256) >> 4) - 8).astype(np.float32) / 7.0,
], axis=1)


def _encode_int4(hf, s):
    """f32 (rows, HW) -> packed nibbles (rows, HW/2), 4-way threaded."""
    import concurrent.futures as cf
    if "pool" not in _ctx:
        _ctx["pool"] = cf.ThreadPoolExecutor(max_workers=4)
    hp = np.empty((hf.shape[0], hf.shape[1] // 2), np.uint8)
    n = hf.shape[0]
    ch = (n + 3) // 4
    def do(i):
        sl = slice(i * ch, min((i + 1) * ch, n))
        q = np.clip(np.rint(hf[sl] * (1.0 / s)), -7, 7).astype(np.int8) + 8
        qq = q.view(np.uint8)
        hp[sl] = qq[:, 0::2] | (qq[:, 1::2] << 4)
    list(_ctx["pool"].map(do, range(4)))
    return hp


def _attention_device(h_):
    """h_: (B, C, HW) float32. Returns h2 (B, C, HW) float32."""
    hf = h_.reshape(B * C, HW)
    s = float(np.abs(hf).max()) / 7.0
    hp = _encode_int4(hf, s)
    sv = np.full((128, 1), s, np.float32)
    bv = (_ctx["b1v"] - (8.0 * s) * _ctx["wsum"]).astype(np.float32)
    raw = np.asarray(_ctx["fn"](hp, sv, bv, *_ctx["dev_w"]))
    # decode: h2 = nib/7 * amax / 16  (device h2n is 16*h2; scale in tail bytes)
    sc = np.ascontiguousarray(raw[:, HW // 2:]).view(np.float32) * (1.0 / 16.0)
    h2 = np.empty((B * C, HW), np.float32)
    n = B * C
    ch = (n + 3) // 4
    def dec(i):
        sl = slice(i * ch, min((i + 1) * ch, n))
        h2[sl] = _I4_LUT[raw[sl, :HW // 2]].reshape(-1, HW)
        h2[sl] *= sc[sl]
    list(_ctx["pool"].map(dec, range(4)))
    return h2.reshape(B, C, HW)


# ---------------- host-side glue (numpy) ----------------

def _softmax(x, axis):
    m = np.max(x, axis=axis, keepdims=True)
    e = np.exp(x - m)
    return e / e.sum(axis=axis, keepdims=True)


def _conv1x1(x, w, b):
    y = np.einsum("oc,bchw->bohw", w[:, :, 0, 0], x, optimize=True)
    return y + b[None, :, None, None]


def _dwconv(x, w, b=None):
    kh, kw = w.shape[2], w.shape[3]
    ph, pw = kh // 2, kw // 2
    xp = np.pad(x, ((0, 0), (0, 0), (ph, ph), (pw, pw)))
    Hh, Wh = x.shape[2], x.shape[3]
    out = np.zeros_like(x)
    for i in range(kh):
        for j in range(kw):
            out += xp[:, :, i : i + Hh, j : j + Wh] * w[None, :, 0, i, j, None, None]
    if b is not None:
        out = out + b[None, :, None, None]
    return out


def _gauss_kernel(ks, sigma, c):
    i = np.arange(ks) - (ks - 1) / 2.0
    g = np.exp(-(i ** 2) / (2.0 * sigma ** 2))
    g = g / g.sum()
    k2 = np.outer(g, g).astype(np.float32)
    return np.broadcast_to(k2[None, None], (c, 1, ks, ks)).copy()


def _group_norm(x, scale, bias):
    b, c, h, w = x.shape
    xg = x.reshape(b, GROUPS, c // GROUPS, h, w)
    mu = xg.mean(axis=(2, 3, 4), keepdims=True, dtype=np.float32)
    var = xg.var(axis=(2, 3, 4), keepdims=True, dtype=np.float32)
    xn = ((xg - mu) / np.sqrt(var + 1e-6)).reshape(b, c, h, w)
    return xn * scale[None, :, None, None] + bias[None, :, None, None]


def _laplacian_attention(x):
    b, c = x.shape[0], x.shape[1]
    L0 = x.reshape(b, c, HW)
    s0 = _softmax(L0, 2)
    att = _softmax(np.matmul(s0, L0.transpose(0, 2, 1)), -1)
    sigma, s = 1.6, 2.0 ** (1.0 / 3.0)
    pyr = [x]
    G = x
    for i in range(2):  # level 3 of the pyramid is computed but unused upstream
        G = _dwconv(G, _gauss_kernel(2 * i + 3, sigma * s ** i, c))
        pyr.append(G)
    for i in range(1, 3):
        L = (pyr[i - 1] - pyr[i]).reshape(b, c, HW)
        att = att + np.matmul(_softmax(L, 2), L.transpose(0, 2, 1))
    return att


def _attention_numpy(h_, q1_w, q1_b, q2_w, q2_b, k1_w, k1_b, k2_w, k2_b,
                     v1_w, v1_b, v2_w, v2_b):
    """Fallback if the device path is unavailable."""
    hi = h_.reshape(B, C, HH, WW)
    q = _dwconv(_conv1x1(hi, q1_w, q1_b), q2_w, q2_b).reshape(B, C, HW)
    k = _dwconv(_conv1x1(hi, k1_w, k1_b), k2_w, k2_b).reshape(B, C, HW)
    v = _dwconv(_conv1x1(hi, v1_w, v1_b), v2_w, v2_b).reshape(B, C, HW)
    h2 = np.empty((B, C, HW), np.float32)
    for b in range(B):
        scores = (q[b].T @ k[b]) * (C ** -0.5)
        attn = _softmax(scores, 1)
        h2[b] = v[b] @ attn.T
    return h2


def kernel(x, gn_scale, gn_bias, q1_w, q1_b, q2_w, q2_b, k1_w, k1_b, k2_w, k2_b,
           v1_w, v1_b, v2_w, v2_b, proj_w, proj_b, mid_w, mid_b, post_w, post_b,
           c1_w, c1_b):
    (gn_scale, gn_bias, q1_w, q1_b, q2_w, q2_b, k1_w, k1_b, k2_w, k2_b, v1_w,
     v1_b, v2_w, v2_b, proj_w, proj_b, mid_w, mid_b, post_w, post_b, c1_w,
     c1_b) = (np.asarray(a, np.float32) for a in (
        gn_scale, gn_bias, q1_w, q1_b, q2_w, q2_b, k1_w, k1_b, k2_w, k2_b,
        v1_w, v1_b, v2_w, v2_b, proj_w, proj_b, mid_w, mid_b, post_w, post_b,
        c1_w, c1_b))
    x = np.asarray(x, np.float32)
    h_ = _group_norm(x, gn_scale, gn_bias)
    hf = h_.reshape(B, C, HW)

    # The phase branch (Laplacian attention -> fa -> rfft2 -> arctan2 ->
    # mid-conv -> cos/sin) needs only x and the host-side qf; it overlaps
    # with the device round trip.
    def _phase_branch():
        qf = _dwconv(_conv1x1(h_, q1_w, q1_b), q2_w, q2_b).reshape(B, C, HW)
        fc = _laplacian_attention(x)
        fa = np.einsum("bji,bjn->bin", fc, qf, optimize=True).reshape(B, C, HH, WW)
        Fd = np.fft.rfft2(fa)
        pha = _dwconv(np.arctan2(Fd.imag, Fd.real).astype(np.float32), mid_w, mid_b)
        return np.cos(pha), np.sin(pha)

    import concurrent.futures as cf
    with cf.ThreadPoolExecutor(max_workers=1) as ex:
        pha_fut = ex.submit(_phase_branch)
        try:
            _setup(_prep_weights(q1_w, q1_b, q2_w, q2_b, k1_w, k1_b, k2_w, k2_b,
                                 v1_w, v1_b, v2_w, v2_b))
            h2 = _attention_device(hf)
        except Exception:
            h2 = _attention_numpy(hf, q1_w, q1_b, q2_w, q2_b, k1_w, k1_b,
                                  k2_w, k2_b, v1_w, v1_b, v2_w, v2_b)
        cosp, sinp = pha_fut.result()

    h2 = _conv1x1(h2.reshape(B, C, HH, WW), proj_w, proj_b)
    Fe = np.fft.rfft2(h2)
    amp = np.abs(Fe).astype(np.float32)
    real = _conv1x1(amp * cosp, post_w, post_b)
    imag = _dwconv(amp * sinp, c1_w, c1_b)
    rec = np.fft.irfft2(real + 1j * imag).astype(np.float32)
    y = x + rec
    out = y + (y - y.mean(axis=(2, 3), keepdims=True, dtype=np.float32))
    return out.astype(np.float32)
